# revision 1
# baseline (speedup 1.0000x reference)
"""Trainium2 Bass kernel for nn_MultiModalFusion (moe_routing).

Strategy:
- Pure data-parallel over 8 cores; host sorts samples by expert label so each
  core sees 4 contiguous expert groups of fixed capacity (static shapes, only
  1 of 4 expert matmuls runs per sample).
- Feature-partitioned ("transposed") layout on device: activations are
  [feature, sample]; all dense math is weight-stationary fp32r matmuls
  (1 cyc/row for N>=256, ~1.5e-4 rel err).
- out_proj is folded into fus_w1 on the host (saves a full [B*3,512]x[512,512]
  matmul); the 1/sqrt(hd) score scale is folded into W_q.
- Tiny-seq (3 tokens) attention: q*k products on DVE (bf16, 2x mode);
  per-head d-reduction and softmax probability broadcast-over-d are done with
  small constant selection-matrix matmuls on the tensor engine.
"""

import numpy as np

import concourse.bass as bass
import concourse.mybir as mybir
import concourse.tile as tile
from concourse import bacc
from concourse.bass_utils import run_bass_kernel_spmd

E = 512
H = 256
NH = 8
HD = 64
NE = 4
B = 16384
NCORES = 8
CAP = 544            # per-core per-expert capacity (ceil(B/4/8) + slack)
R = NE * CAP         # 2176 columns per core
C = 272              # chunk = half an expert group; 8 chunks, all N>=256
NCH = R // C

LAST_RESULTS = None  # BassKernelResults of the most recent kernel() call
LAST_NC = None       # finalized Bass program of the most recent kernel() call

F32 = mybir.dt.float32
F32R = mybir.dt.float32r
BF16 = mybir.dt.bfloat16
AF = mybir.ActivationFunctionType
ALU = mybir.AluOpType


_NC_CACHE = []


def _build_program():
    if _NC_CACHE:
        return _NC_CACHE[0]
    nc = bacc.Bacc("TRN2")

    # ---------------- DRAM I/O ----------------
    xt = nc.dram_tensor("xt", [3, 4, 128, R], F32R, kind="ExternalInput")
    wqkv = nc.dram_tensor("wqkv", [128, 4, 1536], F32R, kind="ExternalInput")
    bqkv = nc.dram_tensor("bqkv", [128, 12], F32, kind="ExternalInput")
    w1o = nc.dram_tensor("w1o", [128, 12, 256], F32R, kind="ExternalInput")
    beff = nc.dram_tensor("beff", [128, 2], F32, kind="ExternalInput")
    w2 = nc.dram_tensor("w2", [128, 2, 512], F32R, kind="ExternalInput")
    b2 = nc.dram_tensor("b2", [128, 4], F32, kind="ExternalInput")
    lng = nc.dram_tensor("lng", [128, 4], F32, kind="ExternalInput")
    lnb = nc.dram_tensor("lnb", [128, 4], F32, kind="ExternalInput")
    waff = nc.dram_tensor("waff", [128, 4, 2048], F32R, kind="ExternalInput")
    baff = nc.dram_tensor("baff", [128, 16], F32, kind="ExternalInput")
    sel = nc.dram_tensor("sel", [128, 4, 8], BF16, kind="ExternalInput")
    exps = nc.dram_tensor("exps", [8, 4, 128], F32R, kind="ExternalInput")
    ones512 = nc.dram_tensor("ones512", [128, 1], F32R, kind="ExternalInput")
    onesk1 = nc.dram_tensor("onesk1", [1, 128], F32R, kind="ExternalInput")
    outT = nc.dram_tensor("outT", [4, 128, R], F32, kind="ExternalOutput")

    with tile.TileContext(nc) as tc:
        with tc.tile_pool(name="wp", bufs=1) as wp, \
             tc.tile_pool(name="xp", bufs=2) as xp, \
             tc.tile_pool(name="ap", bufs=1) as ap, \
             tc.tile_pool(name="ap2", bufs=2) as ap2, \
             tc.tile_pool(name="sp", bufs=1) as sp, \
             tc.tile_pool(name="sp2", bufs=3) as sp2, \
             tc.tile_pool(name="psQ", bufs=3, space="PSUM") as psQ, \
             tc.tile_pool(name="psE", bufs=2, space="PSUM") as psE, \
             tc.tile_pool(name="psH", bufs=1, space="PSUM") as psH, \
             tc.tile_pool(name="psS", bufs=1, space="PSUM") as psS, \
             tc.tile_pool(name="psT", bufs=2, space="PSUM") as psT:

            # ---------------- load weights/constants once ----------------
            wqkv_sb = wp.tile([128, 4, 1536], F32R)
            for ks in range(4):
                nc.sync.dma_start(wqkv_sb[:, ks, :], wqkv[:, ks, :])
            bqkv_sb = wp.tile([128, 12], F32)
            nc.sync.dma_start(bqkv_sb[:], bqkv[:])
            w1o_sb = wp.tile([128, 12, 256], F32R)
            nc.sync.dma_start(w1o_sb[:], w1o[:])
            beff_sb = wp.tile([128, 2], F32)
            nc.sync.dma_start(beff_sb[:], beff[:])
            w2_sb = wp.tile([128, 2, 512], F32R)
            nc.sync.dma_start(w2_sb[:], w2[:])
            b2_sb = wp.tile([128, 4], F32)
            nc.sync.dma_start(b2_sb[:], b2[:])
            lng_sb = wp.tile([128, 4], F32)
            nc.sync.dma_start(lng_sb[:], lng[:])
            lnb_sb = wp.tile([128, 4], F32)
            nc.sync.dma_start(lnb_sb[:], lnb[:])
            baff_sb = wp.tile([128, 16], F32)
            nc.sync.dma_start(baff_sb[:], baff[:])
            sel_sb = wp.tile([128, 4, 8], BF16)
            nc.sync.dma_start(sel_sb[:], sel[:])
            exps_sb = wp.tile([8, 4, 128], F32R)
            nc.sync.dma_start(exps_sb[:], exps[:])
            o512_sb = wp.tile([128, 1], F32R)
            nc.sync.dma_start(o512_sb[:], ones512[:])
            ok1_sb = wp.tile([1, 128], F32R)
            nc.sync.dma_start(ok1_sb[:], onesk1[:])
            eps_sb = wp.tile([1, 1], F32)
            nc.vector.memset(eps_sb[:], 1e-5)

            def front_alloc(ch):
                """x/waff loads + qkv tile allocation for chunk ch."""
                col = ch * C
                exp_idx = ch // 2
                x_sb = xp.tile([128, 3, 4, C], F32R, tag="x",
                               name=f"x{ch}")
                for t in range(3):
                    for p in range(4):
                        nc.sync.dma_start(x_sb[:, t, p, :],
                                          xt[t, p, :, col:col + C])
                q_sb = ap2.tile([128, 3, 4, C], BF16, tag="q", name=f"q{ch}")
                k_sb = ap2.tile([128, 3, 4, C], BF16, tag="k", name=f"k{ch}")
                v_sb = ap2.tile([128, 3, 4, C], F32R, tag="v", name=f"v{ch}")
                waff_sb = ap2.tile([128, 4, 512], F32R, tag="waff",
                                   name=f"waff{ch}")
                nc.sync.dma_start(
                    waff_sb[:],
                    waff[:, :, exp_idx * 512:(exp_idx + 1) * 512])
                return {"ch": ch, "x": x_sb, "q": q_sb, "k": k_sb,
                        "v": v_sb, "waff": waff_sb}

            def front_qkv(st, t):
                """QKV projection for one token of chunk st['ch']."""
                ch = st["ch"]
                x_sb = st["x"]
                for mi in range(12):
                    qp = psQ.tile([128, C], F32, tag="qkv",
                                  name=f"qkv{ch}_{t}_{mi}")
                    for ks in range(4):
                        nc.tensor.matmul(
                            qp[:],
                            wqkv_sb[:, ks, mi * 128:(mi + 1) * 128],
                            x_sb[:, t, ks, :],
                            start=(ks == 0), stop=(ks == 3))
                    dst = (st["q"], st["k"], st["v"])[mi // 4]
                    nc.scalar.activation(
                        dst[:, t, mi % 4, :], qp[:], AF.Identity,
                        bias=bqkv_sb[:, mi:mi + 1], scale=1.0)

            def stage_back(ch, q_sb, k_sb, v_sb, waff_sb, interleave):
                """attention middle + MLP + LN + expert for chunk ch.
                `interleave` is a list of thunks (next chunk's per-token QKV)
                issued between phases so the static PE stream stays dense."""
                col = ch * C
                exp_idx = ch // 2

                # ---------------- attention scores ----------------
                e_sb = sp.tile([8, 3, 3, C], F32R, tag="esb",
                               name=f"e{ch}")
                for i in range(3):
                    for j in range(3):
                        prod = sp2.tile([128, 4, C], BF16, tag="prod",
                                        name=f"prod{ch}_{i}_{j}")
                        nc.vector.tensor_tensor(
                            prod[:], q_sb[:, i, :, :], k_sb[:, j, :, :],
                            ALU.mult)
                        s_ps = psE.tile([8, C], F32, tag="mid",
                                        name=f"s{ch}_{i}_{j}")
                        for p in range(4):
                            nc.tensor.matmul(
                                s_ps[:], sel_sb[:, p, :], prod[:, p, :],
                                start=(p == 0), stop=(p == 3))
                        nc.scalar.activation(
                            e_sb[:, i, j, :], s_ps[:], AF.Exp)

                # ---------------- softmax over j ----------------
                e_f32 = e_sb.bitcast(F32)
                z_sb = sp.tile([8, 3, C], F32, tag="z")
                nc.vector.tensor_tensor(z_sb[:], e_f32[:, :, 0, :],
                                        e_f32[:, :, 1, :], ALU.add)
                nc.vector.tensor_tensor(z_sb[:], z_sb[:], e_f32[:, :, 2, :],
                                        ALU.add)
                nc.vector.reciprocal_approx_fast(z_sb[:], z_sb[:])
                p_sb = e_sb
                nc.vector.tensor_tensor(
                    p_sb[:], e_f32[:],
                    z_sb[:, :, None, :].to_broadcast((8, 3, 3, C)), ALU.mult)
                if len(interleave) > 0:
                    interleave[0]()

                # ------------- weighted sum over j (via PE broadcast) -------------
                o_sb = ap.tile([128, 12, C], F32R, tag="o")
                for i in range(3):
                    pvi = ap.tile([128, 4, 3, C], F32, tag="pv",
                                  name=f"pv{ch}_{i}")
                    for p in range(4):
                        for j in range(3):
                            pe_ps = psE.tile([128, C], F32, tag="mid",
                                             name=f"pe{ch}_{i}_{p}_{j}")
                            nc.tensor.matmul(
                                pe_ps[:], exps_sb[:, p, :],
                                p_sb[:, i, j, :], start=True, stop=True)
                            nc.vector.tensor_tensor(
                                pvi[:, p, j, :], pe_ps[:], v_sb[:, j, p, :],
                                ALU.mult)
                    nc.vector.tensor_tensor(pvi[:, :, 0, :], pvi[:, :, 0, :],
                                            pvi[:, :, 1, :], ALU.add)
                    nc.vector.tensor_tensor(o_sb[:, i * 4:(i + 1) * 4, :],
                                            pvi[:, :, 0, :], pvi[:, :, 2, :],
                                            ALU.add)
                if len(interleave) > 1:
                    interleave[1]()

                # ---------------- fused W1(out_proj .) + ReLU ----------------
                hpre_sb = ap.tile([128, 2, C], F32R, tag="hpre")
                for m2 in range(2):
                    hp = psH.tile([128, C], F32, tag="tail", name=f"hp{ch}_{m2}")
                    for kip in range(12):
                        nc.tensor.matmul(
                            hp[:], w1o_sb[:, kip, m2 * 128:(m2 + 1) * 128],
                            o_sb[:, kip, :],
                            start=(kip == 0), stop=(kip == 11))
                    nc.scalar.activation(hpre_sb[:, m2, :], hp[:], AF.Relu,
                                         bias=beff_sb[:, m2:m2 + 1], scale=1.0)

                # ---------------- fus2 ----------------
                y_sb = ap.tile([128, 4, C], F32R, tag="y")
                for m4 in range(4):
                    yp = psH.tile([128, C], F32, tag="tail", name=f"yp{ch}_{m4}")
                    for ks in range(2):
                        nc.tensor.matmul(
                            yp[:], w2_sb[:, ks, m4 * 128:(m4 + 1) * 128],
                            hpre_sb[:, ks, :], start=(ks == 0), stop=(ks == 1))
                    nc.scalar.activation(y_sb[:, m4, :], yp[:], AF.Identity,
                                         bias=b2_sb[:, m4:m4 + 1], scale=1.0)
                if len(interleave) > 2:
                    interleave[2]()

                # ---------------- LayerNorm ----------------
                mu_ps = psT.tile([1, C], F32, tag="st", name=f"mu{ch}")
                for p in range(4):
                    nc.tensor.matmul(mu_ps[:], o512_sb[:], y_sb[:, p, :],
                                     start=(p == 0), stop=(p == 3))
                m2_ps = psT.tile([1, C], F32, tag="st", name=f"m2{ch}")
                for p in range(4):
                    ysq = sp2.tile([128, C], F32R, tag="ysq",
                                   name=f"ysq{ch}_{p}")
                    nc.scalar.activation(ysq[:], y_sb[:, p, :], AF.Square)
                    nc.tensor.matmul(m2_ps[:], o512_sb[:], ysq[:],
                                     start=(p == 0), stop=(p == 3))
                mu_sb = sp.tile([1, C], F32R, tag="musb")
                nc.scalar.copy(mu_sb[:], mu_ps[:])
                var_sb = sp.tile([1, C], F32, tag="varsb")
                # var = E[y^2] - mu^2  (psum m2 minus mu*mu)
                musq = sp.tile([1, C], F32, tag="musq")
                nc.vector.tensor_tensor(musq[:], mu_sb.bitcast(F32)[:],
                                        mu_sb.bitcast(F32)[:], ALU.mult)
                nc.vector.tensor_tensor(var_sb[:], m2_ps[:], musq[:],
                                        ALU.subtract)
                sd_sb = sp.tile([1, C], F32, tag="sdsb")
                nc.scalar.activation(sd_sb[:], var_sb[:], AF.Sqrt,
                                     bias=eps_sb[:], scale=1.0)
                rstd_f = sp.tile([1, C], F32, tag="rstdf")
                nc.vector.reciprocal_approx_fast(rstd_f[:], sd_sb[:])
                rstd_sb = sp.tile([1, C], F32R, tag="rstdsb")
                nc.scalar.copy(rstd_sb[:], rstd_f[:])
                muex_ps = psT.tile([128, C], F32, tag="st", name=f"muex{ch}")
                nc.tensor.matmul(muex_ps[:], ok1_sb[:], mu_sb[:],
                                 start=True, stop=True)
                rsex_ps = psT.tile([128, C], F32, tag="st", name=f"rsex{ch}")
                nc.tensor.matmul(rsex_ps[:], ok1_sb[:], rstd_sb[:],
                                 start=True, stop=True)
                fused = ap.tile([128, 4, C], F32R, tag="fused")
                for p in range(4):
                    lnp = sp2.tile([128, C], F32, tag="lnp",
                                   name=f"lnp{ch}_{p}")
                    nc.vector.tensor_tensor(lnp[:], y_sb.bitcast(F32)[:, p, :],
                                            muex_ps[:], ALU.subtract)
                    nc.vector.tensor_tensor(lnp[:], lnp[:], rsex_ps[:],
                                            ALU.mult)
                    nc.scalar.activation(fused[:, p, :], lnp[:],
                                         AF.Identity,
                                         bias=lnb_sb[:, p:p + 1],
                                         scale=lng_sb[:, p:p + 1])

                # ---------------- routed expert matmul ----------------
                for m4 in range(4):
                    op = psH.tile([128, C], F32, tag="tail", name=f"op{ch}_{m4}")
                    for ks in range(4):
                        nc.tensor.matmul(
                            op[:],
                            waff_sb[:, ks, m4 * 128:(m4 + 1) * 128],
                            fused[:, ks, :], start=(ks == 0), stop=(ks == 3))
                    ot = sp2.tile([128, C], F32, tag="ot", name=f"ot{ch}_{m4}")
                    nc.scalar.activation(
                        ot[:], op[:], AF.Identity,
                        bias=baff_sb[:, exp_idx * 4 + m4:exp_idx * 4 + m4 + 1],
                        scale=1.0)
                    nc.sync.dma_start(outT[m4, :, col:col + C], ot[:])

            cur = front_alloc(0)
            for t in range(3):
                front_qkv(cur, t)
            for ch in range(NCH):
                if ch + 1 < NCH:
                    nxt = front_alloc(ch + 1)
                    il = [lambda t=t, s=nxt: front_qkv(s, t)
                          for t in range(3)]
                else:
                    nxt, il = None, []
                stage_back(ch, cur["q"], cur["k"], cur["v"], cur["waff"], il)
                cur = nxt

    nc.finalize()
    _NC_CACHE.append(nc)
    return nc


def _prep_weights(inputs):
    in_proj_w = np.asarray(inputs["in_proj_w"], np.float32)
    in_proj_b = np.asarray(inputs["in_proj_b"], np.float32)
    out_proj_w = np.asarray(inputs["out_proj_w"], np.float32)
    out_proj_b = np.asarray(inputs["out_proj_b"], np.float32)
    fus_w1 = np.asarray(inputs["fus_w1"], np.float32)
    fus_b1 = np.asarray(inputs["fus_b1"], np.float32)
    fus_w2 = np.asarray(inputs["fus_w2"], np.float32)
    fus_b2 = np.asarray(inputs["fus_b2"], np.float32)
    ln_g = np.asarray(inputs["ln_g"], np.float32)
    ln_b = np.asarray(inputs["ln_b"], np.float32)
    aff_w = np.asarray(inputs["aff_w"], np.float32)
    aff_b = np.asarray(inputs["aff_b"], np.float32)

    scale = 1.0 / np.sqrt(np.float32(HD))
    W = in_proj_w.copy()
    W[:E] *= scale
    bq = in_proj_b.copy()
    bq[:E] *= scale
    # W.T is [512(k), 1536(m)]; sbuf wants [128, 4(ksub), 1536]
    wqkv_h = np.ascontiguousarray(
        W.T.reshape(4, 128, 1536).transpose(1, 0, 2))
    bqkv_h = np.ascontiguousarray(bq.reshape(12, 128).T)

    # fold out_proj into fus_w1; permute (h,d) -> (p, hl, d) to match v layout
    perm = np.empty(E, np.int64)
    for h in range(NH):
        for d in range(HD):
            perm[(h // 2) * 128 + (h % 2) * 64 + d] = h * HD + d
    blocks = []
    for i in range(3):
        blk = fus_w1[:, i * E:(i + 1) * E] @ out_proj_w  # [256, 512]
        blocks.append(blk[:, perm])
    W1o = np.concatenate(blocks, axis=1)  # [256, 1536] cols = (i, p, hl, d)
    w1o_h = np.ascontiguousarray(W1o.T.reshape(12, 128, 256).transpose(1, 0, 2))
    beff = fus_b1 + fus_w1 @ np.tile(out_proj_b, 3)
    beff_h = np.ascontiguousarray(beff.reshape(2, 128).T)

    w2_h = np.ascontiguousarray(fus_w2.T.reshape(2, 128, 512).transpose(1, 0, 2))
    b2_h = np.ascontiguousarray(fus_b2.reshape(4, 128).T)
    lng_h = np.ascontiguousarray(ln_g.reshape(4, 128).T)
    lnb_h = np.ascontiguousarray(ln_b.reshape(4, 128).T)

    A = np.concatenate([aff_w[e].T for e in range(NE)], axis=1)  # [512, 2048]
    waff_h = np.ascontiguousarray(A.reshape(4, 128, 2048).transpose(1, 0, 2))
    baff_h = np.ascontiguousarray(aff_b.reshape(NE * 4, 128).T)

    sel_h = np.zeros((128, 4, 8), np.float32)
    for r in range(128):
        for p in range(4):
            sel_h[r, p, 2 * p + r // 64] = 1.0
    exps_h = np.zeros((8, 4, 128), np.float32)
    for p in range(4):
        for c in range(128):
            exps_h[2 * p + c // 64, p, c] = 1.0

    bf = mybir.dt.np(BF16)
    return {
        "wqkv": wqkv_h, "bqkv": bqkv_h, "w1o": w1o_h, "beff": beff_h,
        "w2": w2_h, "b2": b2_h, "lng": lng_h, "lnb": lnb_h,
        "waff": waff_h, "baff": baff_h,
        "sel": sel_h.astype(bf), "exps": exps_h,
        "ones512": np.full((128, 1), 1.0 / E, np.float32),
        "onesk1": np.ones((1, 128), np.float32),
    }


def kernel(**inputs):
    img = np.asarray(inputs["image_embeddings"], np.float32)
    txt = np.asarray(inputs["text_embeddings"], np.float32)
    kno = np.asarray(inputs["knowledge_embeddings"], np.float32)
    labels = np.asarray(inputs["affective_labels"]).astype(np.int64).ravel()
    assert img.shape == (B, E)

    # ---- host-side expert routing (per-core fixed capacities) ----
    core_idx = np.zeros((NCORES, R), np.int64)
    core_val = np.zeros((NCORES, R), bool)
    for e in range(NE):
        ids = np.nonzero(labels == e)[0]
        assert len(ids) <= NCORES * CAP, f"expert {e} overflow: {len(ids)}"
        parts = np.array_split(ids, NCORES)
        for c in range(NCORES):
            seg = parts[c]
            core_idx[c, e * CAP: e * CAP + len(seg)] = seg
            core_val[c, e * CAP: e * CAP + len(seg)] = True

    wmap = _prep_weights(inputs)

    in_maps = []
    for c in range(NCORES):
        gi = core_idx[c]
        xg = np.stack([img[gi], txt[gi], kno[gi]])        # [3, R, 512]
        xg = xg.transpose(0, 2, 1)                        # [3, 512, R]
        xt_h = np.ascontiguousarray(xg.reshape(3, 4, 128, R))
        m = dict(wmap)
        m["xt"] = xt_h
        in_maps.append(m)

    nc = _build_program()
    res = run_bass_kernel_spmd(nc, in_maps, core_ids=list(range(NCORES)))
    global LAST_RESULTS, LAST_NC
    LAST_RESULTS = res
    LAST_NC = nc

    out_full = np.zeros((B, E), np.float32)
    for c in range(NCORES):
        oT = res.results[c]["outT"].reshape(E, R).T       # [R, 512]
        v = core_val[c]
        out_full[core_idx[c][v]] = oT[v]
    return out_full


if __name__ == "__main__":
    rng = np.random.default_rng(0)
    fake = {
        "image_embeddings": rng.standard_normal((B, E)).astype(np.float32),
        "text_embeddings": rng.standard_normal((B, E)).astype(np.float32),
        "knowledge_embeddings": rng.standard_normal((B, E)).astype(np.float32),
        "affective_labels": rng.integers(0, NE, B),
        "in_proj_w": (rng.standard_normal((3 * E, E)) * 0.02).astype(np.float32),
        "in_proj_b": np.zeros(3 * E, np.float32),
        "out_proj_w": (rng.standard_normal((E, E)) * 0.02).astype(np.float32),
        "out_proj_b": np.zeros(E, np.float32),
        "fus_w1": (rng.standard_normal((H, 3 * E)) * 0.02).astype(np.float32),
        "fus_b1": np.zeros(H, np.float32),
        "fus_w2": (rng.standard_normal((E, H)) * 0.02).astype(np.float32),
        "fus_b2": np.zeros(E, np.float32),
        "ln_g": np.ones(E, np.float32),
        "ln_b": np.zeros(E, np.float32),
        "aff_w": (rng.standard_normal((NE, E, E)) * 0.02).astype(np.float32),
        "aff_b": np.zeros((NE, E), np.float32),
    }
    out = kernel(**fake)
    print("kernel ran, out:", out.shape, out.dtype, np.abs(out).max())



# revision 86
# speedup vs baseline: 1.3075x; 1.3075x over previous
"""Trainium2 Bass kernel for nn_MultiModalFusion (moe_routing).

Strategy:
- Pure data-parallel over 8 cores; host sorts samples by expert label so each
  core sees 4 contiguous expert groups of fixed capacity (static shapes, only
  1 of 4 expert matmuls runs per sample).
- Feature-partitioned ("transposed") layout on device: activations are
  [feature, sample]; dense math is weight-stationary bf16 matmuls.
- Host folds: out_proj into fus_w1; 1/sqrt(hd) into W_q; ln_g into the expert
  weights; ln_b/aff_b into the expert bias.
- Attention: q*k products on DVE/GpSimd (bf16 2x); per-head d-reduction fused
  with the head->feature broadcast via one block-diagonal selection matmul
  per plane; softmax runs on the broadcast domain in bf16 (exp on Act from
  2-bank psum tiles, z-sums on GpSimd, reciprocal_approx_fast + weighted sum
  on DVE; TT-divide is not supported by the DVE ISA).
- LayerNorm: mean/E[y^2] via 1/512-vector matmuls; rstd via the bit-trick
  inverse sqrt + one Newton step on DVE (keeps Ln/Sqrt off the Act engine so
  a single act-func table serves the kernel - no table-swap stalls); mu/rstd
  broadcast to 128 partitions via gpsimd.partition_broadcast.
- Engine balance: PSUM evacuation split DVE/Act (GpSimd has no PSUM port);
  elementwise bf16 on DVE; GpSimd carries z-sums, 3 of 9 q*k products, and
  the partition broadcasts.
- Software pipeline: chunk N+1 QKV m-tiles are pumped between chunk N back
  phases from a cross-chunk work queue (barrier keeps in-order engines
  consistent); the last chunk reserves its v-tiles as its own pump material.
- Startup: wqkv streams on the Activation HWDGE queue in parallel with
  x/bias on the sync queue; first matmul issues ~4us in.
"""

import numpy as np

import concourse.bass as bass
import concourse.mybir as mybir
import concourse.tile as tile
from concourse import bacc
from concourse.bass_utils import run_bass_kernel_spmd

E = 512
H = 256
NH = 8
HD = 64
NE = 4
B = 16384
NCORES = 8
CAP = 544            # per-core per-expert capacity (ceil(B/4/8) + slack)
R = NE * CAP         # 2176 columns per core
C = 272              # chunk = half an expert group; 8 chunks, all N>=256
NCH = R // C

LAST_RESULTS = None  # BassKernelResults of the most recent kernel() call
LAST_NC = None       # finalized Bass program of the most recent kernel() call

F32 = mybir.dt.float32
F32R = mybir.dt.float32r
BF16 = mybir.dt.bfloat16
AF = mybir.ActivationFunctionType
ALU = mybir.AluOpType


_NC_CACHE = {}


def _build_program(key=()):
    if key in _NC_CACHE:
        return _NC_CACHE[key]
    nc = bacc.Bacc("TRN2")

    # ---------------- DRAM I/O ----------------
    xt = nc.dram_tensor("xt", [3, 4, 128, R], BF16, kind="ExternalInput")
    wqkv = nc.dram_tensor("wqkv", [128, 4, 1536], BF16, kind="ExternalInput")
    bqkv = nc.dram_tensor("bqkv", [128, 12], F32, kind="ExternalInput")
    w1o = nc.dram_tensor("w1o", [128, 12, 256], BF16, kind="ExternalInput")
    beff = nc.dram_tensor("beff", [128, 2], F32, kind="ExternalInput")
    w2 = nc.dram_tensor("w2", [128, 2, 512], BF16, kind="ExternalInput")
    b2 = nc.dram_tensor("b2", [128, 4], F32, kind="ExternalInput")
    waff = nc.dram_tensor("waff", [128, 4, 2048], BF16, kind="ExternalInput")
    baff = nc.dram_tensor("baff", [128, 16], F32, kind="ExternalInput")
    sel2 = nc.dram_tensor("sel2", [128, 128], BF16, kind="ExternalInput")
    ones512 = nc.dram_tensor("ones512", [128, 1], BF16, kind="ExternalInput")
    outT = nc.dram_tensor("outT", [4, 128, R], F32, kind="ExternalOutput")

    with tile.TileContext(nc) as tc:
        with tc.tile_pool(name="wp", bufs=1) as wp, \
             tc.tile_pool(name="xp", bufs=2) as xp, \
             tc.tile_pool(name="qkvp", bufs=2) as qkvp, \
             tc.tile_pool(name="wap", bufs=2) as wap, \
             tc.tile_pool(name="ep", bufs=1) as ep, \
             tc.tile_pool(name="sp", bufs=2) as sp, \
             tc.tile_pool(name="sp2", bufs=3) as sp2, \
             tc.tile_pool(name="op", bufs=1) as op_pool, \
             tc.tile_pool(name="psQ", bufs=3, space="PSUM") as psQ, \
             tc.tile_pool(name="psS", bufs=2, space="PSUM") as psS, \
             tc.tile_pool(name="psH", bufs=1, space="PSUM") as psH:

            def front_alloc(ch, xts=None):
                """x/waff loads + q/k/v/e tile allocation for chunk ch."""
                col = ch * C
                exp_idx = ch // 2
                if xts is None:
                    xts = []
                    for t in range(3):
                        x_t = xp.tile([128, 4, C], BF16, tag=f"x{t}",
                                      name=f"x{ch}_{t}")
                        for p in range(4):
                            nc.sync.dma_start(x_t[:, p, :],
                                              xt[t, p, :, col:col + C])
                        xts.append(x_t)
                q_sb = qkvp.tile([128, 3, 4, C], BF16, tag="q", name=f"q{ch}")
                k_sb = qkvp.tile([128, 3, 4, C], BF16, tag="k", name=f"k{ch}")
                v_sb = qkvp.tile([128, 3, 4, C], BF16, tag="v", name=f"v{ch}")
                e_sb = ep.tile([128, 3, 3, 4, C], BF16, tag="esb",
                               name=f"e{ch}")
                if ch % 2 == 0:
                    waff_sb = wap.tile([128, 4, 512], BF16, tag="waff",
                                       name=f"waff{ch}")
                    nc.sync.dma_start(
                        waff_sb[:],
                        waff[:, :, exp_idx * 512:(exp_idx + 1) * 512])
                else:
                    waff_sb = None
                return {"ch": ch, "x": xts, "q": q_sb, "k": k_sb,
                        "v": v_sb, "e": e_sb, "waff": waff_sb}

            def qkv_mtile(st, t, mi):
                """One QKV output tile: 4 matmuls + Pool evacuation w/ bias."""
                ch = st["ch"]
                qp = psQ.tile([128, C], F32, tag="qkv",
                              name=f"qkv{ch}_{t}_{mi}")
                wq = wqkv_sb[mi // 3]
                msub = mi % 3
                for ks in range(4):
                    nc.tensor.matmul(
                        qp[:],
                        wq[:, ks, msub * 128:(msub + 1) * 128],
                        st["x"][t][:, ks, :],
                        start=(ks == 0), stop=(ks == 3))
                dst = (st["q"], st["k"], st["v"])[mi // 4]
                if phase[0] == 0:
                    use_dve = True          # scores phase: Act runs exp
                elif phase[0] == 1:
                    use_dve = False         # ZPV phase: DVE runs pv
                else:
                    evac_alt[0] ^= 1
                    use_dve = bool(evac_alt[0])
                if use_dve:
                    nc.vector.tensor_scalar_add(
                        dst[:, t, mi % 4, :], qp[:], bqkv_sb[:, mi:mi + 1])
                else:
                    nc.scalar.activation(
                        dst[:, t, mi % 4, :], qp[:], AF.Identity,
                        bias=bqkv_sb[:, mi:mi + 1], scale=1.0)

            def scores_group(st, i, j):
                """q_i*k_j product, head-reduce+broadcast matmuls, exp."""
                ch = st["ch"]
                prod = sp2.tile([128, 4, C], BF16, tag="prod",
                                name=f"prod{ch}_{i}_{j}")
                eng = nc.gpsimd if j == 0 else nc.vector
                eng.tensor_tensor(
                    prod[:], st["q"][:, i, :, :], st["k"][:, j, :, :],
                    ALU.mult)
                for half in range(2):
                    s2 = psS.tile([128, 2, 512], F32, tag="sb",
                                  name=f"s{ch}_{i}_{j}_{half}")
                    for pp in range(2):
                        p = half * 2 + pp
                        nc.tensor.matmul(
                            s2[:, pp, :C], sel2_sb[:],
                            prod[:, p, :], start=True, stop=True)
                    nc.scalar.activation(
                        st["e"][:, i, j, 2 * half:2 * half + 2, :],
                        s2[:, :, :C], AF.Exp)

            queue = []
            states = {}
            phase = [2]
            evac_alt = [0]

            # startup: wqkv on the scalar HWDGE queue (Act idle now) in
            # parallel with bias+x0 on the sync queue, so the first m-tile
            # starts ~3us in and PE is never DMA-starved afterwards
            wqkv_sb = []
            for q4 in range(4):
                wq_t = wp.tile([128, 4, 384], BF16, name=f"wqkv{q4}")
                for ks in range(4):
                    nc.scalar.dma_start(wq_t[:, ks, :],
                                        wqkv[:, ks, q4 * 384:(q4 + 1) * 384])
                wqkv_sb.append(wq_t)
            cur_x = []
            for t in range(3):
                x_t = xp.tile([128, 4, C], BF16, tag=f"x{t}", name=f"x0_{t}")
                for p in range(4):
                    nc.sync.dma_start(x_t[:, p, :], xt[t, p, :, 0:C])
                cur_x.append(x_t)
            bqkv_sb = wp.tile([128, 12], F32)
            nc.sync.dma_start(bqkv_sb[:], bqkv[:])
            cur = front_alloc(0, xts=cur_x)
            states[0] = cur
            w1o_sb = wp.tile([128, 12, 256], BF16)
            nc.sync.dma_start(w1o_sb[:], w1o[:])
            beff_sb = wp.tile([128, 2], F32)
            nc.sync.dma_start(beff_sb[:], beff[:])
            w2_sb = wp.tile([128, 2, 512], BF16)
            nc.sync.dma_start(w2_sb[:], w2[:])
            b2_sb = wp.tile([128, 4], F32)
            nc.sync.dma_start(b2_sb[:], b2[:])
            baff_sb = wp.tile([128, 16], F32)
            nc.sync.dma_start(baff_sb[:], baff[:])
            sel2_sb = wp.tile([128, 128], BF16)
            nc.sync.dma_start(sel2_sb[:], sel2[:])
            o512_sb = wp.tile([128, 1], BF16)
            nc.sync.dma_start(o512_sb[:], ones512[:])
            eps_sb = wp.tile([1, 1], F32)
            nc.vector.memset(eps_sb[:], 1e-5)
            I32 = mybir.dt.int32
            magic_sb = wp.tile([1, C], I32)
            nc.vector.memset(magic_sb[:], 0x5F3759DF)

            def stage_back(ch, st, pump):
                """scores + softmax-weighted sum + MLP + LN + expert for
                chunk ch. `pump(n)` issues n pending next-chunk QKV m-tiles
                to keep the static PE stream dense."""
                col = ch * C
                exp_idx = ch // 2
                v_sb, e_sb, waff_sb = st["v"], st["e"], st["waff"]

                phase[0] = 0
                for i in range(3):
                    for j in range(3):
                        scores_group(st, i, j)
                        pump(3)
                phase[0] = 1
                tail_n = [0]

                def tail_tile(nm):
                    tail_n[0] += 1
                    pool = (psQ if (ch == NCH - 1 and tail_n[0] % 2 == 0)
                            else psH)
                    return pool.tile([128, C], F32, tag="tail" if pool is psH
                                     else "qkv", name=nm)

                # ----- softmax-weighted sum (bf16 pv, f32 z+recip) ----
                # i=0 runs entirely on DVE so o0 lands early (W1's first
                # k-steps read o0); z/o-mult for i=1,2 ride the idler Pool
                o_sb = [op_pool.tile([128, 4, C], BF16, tag=f"o{i}",
                                     name=f"o{ch}_{i}") for i in range(3)]
                zs = []
                for i in range(3):
                    z = sp.tile([128, 4, C], F32, tag=f"z{i}",
                                name=f"z{ch}_{i}", bufs=1)
                    zeng = nc.gpsimd
                    zeng.tensor_tensor(z[:], e_sb[:, i, 0, :, :],
                                       e_sb[:, i, 1, :, :], ALU.add)
                    zeng.tensor_tensor(z[:], z[:], e_sb[:, i, 2, :, :],
                                       ALU.add)
                    zs.append(z)
                for i in range(3):
                    acc = sp.tile([128, 4, C], BF16, tag="acc",
                                  name=f"acc{ch}_{i}")
                    nc.vector.tensor_tensor(acc[:], e_sb[:, i, 0, :, :],
                                            v_sb[:, 0, :, :], ALU.mult)
                    t0 = sp.tile([128, 4, C], BF16, tag="t0",
                                 name=f"t{ch}_{i}")
                    nc.vector.tensor_tensor(t0[:], e_sb[:, i, 1, :, :],
                                            v_sb[:, 1, :, :], ALU.mult)
                    nc.vector.tensor_tensor(acc[:], acc[:], t0[:], ALU.add)
                    pump(1)
                    nc.vector.tensor_tensor(t0[:], e_sb[:, i, 2, :, :],
                                            v_sb[:, 2, :, :], ALU.mult)
                    nc.vector.tensor_tensor(acc[:], acc[:], t0[:], ALU.add)
                    rz = sp.tile([128, 4, C], F32, tag="rz",
                                 name=f"rz{ch}_{i}", bufs=2)
                    nc.vector.reciprocal_approx_fast(rz[:], zs[i][:])
                    oeng = nc.vector
                    oeng.tensor_tensor(o_sb[i][:], acc[:], rz[:], ALU.mult)
                    pump(2)

                phase[0] = 2

                # ---------------- fused W1(out_proj .) + ReLU ----------------
                hpre_sb = sp.tile([128, 2, C], BF16, tag="hpre",
                                  name=f"hpre{ch}")
                for m2 in range(2):
                    hp = tail_tile(f"hp{ch}_{m2}")
                    for kip in range(12):
                        nc.tensor.matmul(
                            hp[:], w1o_sb[:, kip, m2 * 128:(m2 + 1) * 128],
                            o_sb[kip // 4][:, kip % 4, :],
                            start=(kip == 0), stop=(kip == 11))
                    nc.scalar.activation(hpre_sb[:, m2, :], hp[:], AF.Relu,
                                         bias=beff_sb[:, m2:m2 + 1], scale=1.0)
                    pump(2)

                # ---------------- fus2 ----------------
                y_sb = sp.tile([128, 4, C], BF16, tag="y", name=f"y{ch}")
                for m4 in range(4):
                    yp = tail_tile(f"yp{ch}_{m4}")
                    for ks in range(2):
                        nc.tensor.matmul(
                            yp[:], w2_sb[:, ks, m4 * 128:(m4 + 1) * 128],
                            hpre_sb[:, ks, :], start=(ks == 0), stop=(ks == 1))
                    nc.scalar.activation(y_sb[:, m4, :], yp[:], AF.Identity,
                                         bias=b2_sb[:, m4:m4 + 1], scale=1.0)
                    pump(2)

                # ---------------- LayerNorm (stats via bcast, no psum) -------
                mu_ps = psH.tile([1, C], F32, tag="tail", name=f"mu{ch}")
                for p in range(4):
                    nc.tensor.matmul(mu_ps[:], o512_sb[:], y_sb[:, p, :],
                                     start=(p == 0), stop=(p == 3))
                mu_sb = sp.tile([1, C], BF16, tag="musb", name=f"mus{ch}")
                nc.scalar.copy(mu_sb[:], mu_ps[:])
                pump(2)
                ysq_sb = sp.tile([128, 4, C], BF16, tag="ysq",
                                 name=f"ysq{ch}")
                nc.vector.tensor_tensor(ysq_sb[:], y_sb[:], y_sb[:], ALU.mult)
                m2_ps = psH.tile([1, C], F32, tag="tail", name=f"m2{ch}")
                for p in range(4):
                    nc.tensor.matmul(m2_ps[:], o512_sb[:], ysq_sb[:, p, :],
                                     start=(p == 0), stop=(p == 3))
                pump(2)
                musq = sp.tile([1, C], F32, tag="musq", name=f"musq{ch}")
                nc.vector.tensor_tensor(musq[:], mu_sb[:], mu_sb[:], ALU.mult)
                var_sb = sp.tile([1, C], F32, tag="varsb", name=f"var{ch}")
                nc.vector.tensor_tensor(var_sb[:], m2_ps[:], musq[:],
                                        ALU.subtract)
                # rstd = 1/sqrt(var) via the bit-trick seed + one Newton
                # step (~0.17% max err, under the bf16 noise floor). Keeps
                # Ln/Sqrt off the Act engine so one act-func table serves the
                # whole kernel (no 1.3us table swaps). eps=1e-5 is dropped:
                # var ~ 0.2 so it shifts rstd by <3e-5 relative.
                y0i = sp.tile([1, C], I32, tag="y0i", name=f"y0i{ch}")
                nc.vector.tensor_scalar(y0i[:], var_sb.bitcast(I32)[:],
                                        1, None, op0=ALU.arith_shift_right)
                nc.vector.tensor_tensor(y0i[:], magic_sb[:], y0i[:],
                                        ALU.subtract)
                y0f = y0i.bitcast(F32)
                tq = sp.tile([1, C], F32, tag="tq", name=f"tq{ch}")
                nc.vector.tensor_tensor(tq[:], var_sb[:], y0f[:], ALU.mult)
                nc.vector.tensor_tensor(tq[:], tq[:], y0f[:], ALU.mult)
                nc.vector.tensor_scalar(tq[:], tq[:], -0.5, 1.5,
                                        op0=ALU.mult, op1=ALU.add)
                rstd_sb = sp.tile([1, C], BF16, tag="rstd", name=f"rstd{ch}")
                nc.vector.tensor_tensor(rstd_sb[:], y0f[:], tq[:], ALU.mult)
                muex = sp.tile([128, C], BF16, tag="muex", name=f"muex{ch}")
                nc.gpsimd.partition_broadcast(muex[:], mu_sb[:])
                rsex = sp.tile([128, C], BF16, tag="rsex", name=f"rsex{ch}")
                nc.gpsimd.partition_broadcast(rsex[:], rstd_sb[:])
                pump(2)
                fused = sp.tile([128, 4, C], BF16, tag="fused",
                                name=f"fused{ch}")
                t1 = sp.tile([128, 4, C], BF16, tag="t1", name=f"t1{ch}")
                nc.vector.tensor_tensor(
                    t1[:], y_sb[:],
                    muex[:, None, :].to_broadcast((128, 4, C)), ALU.subtract)
                nc.vector.tensor_tensor(
                    fused[:], t1[:],
                    rsex[:, None, :].to_broadcast((128, 4, C)), ALU.mult)
                pump(2)

                # ---------------- routed expert matmul ----------------
                for m4 in range(4):
                    opp = tail_tile(f"op{ch}_{m4}")
                    for ks in range(4):
                        nc.tensor.matmul(
                            opp[:],
                            waff_sb[:, ks, m4 * 128:(m4 + 1) * 128],
                            fused[:, ks, :], start=(ks == 0), stop=(ks == 3))
                    ot = sp2.tile([128, C], F32, tag="ot",
                                  name=f"ot{ch}_{m4}")
                    nc.scalar.activation(
                        ot[:], opp[:], AF.Identity,
                        bias=baff_sb[:, exp_idx * 4 + m4:exp_idx * 4 + m4 + 1],
                        scale=1.0)
                    nc.sync.dma_start(outT[m4, :, col:col + C], ot[:])
                    pump(3)

            # ---------------- main pipeline ----------------
            # Global work queue of (chunk, kind, a, b) closures; in-order
            # engines require every scores group of chunk N to issue before
            # stage_back(N)'s first DVE op, enforced by drain barriers.

            def run_item(item):
                ch_i, kind, a, b = item
                st_i = states[min(ch_i, NCH - 1)]
                if kind == 0:
                    qkv_mtile(st_i, a, b)
                else:
                    scores_group(st_i, a, b)

            pump_limit = [0]

            def pump(n):
                for _ in range(n):
                    if not queue or queue[0][0] > pump_limit[0]:
                        return
                    run_item(queue.pop(0))

            # qkv m-tiles interleaved with score groups as soon as their
            # q_i/k_j dependencies are met, so PE-cheap/Act-heavy score
            # groups never bunch up
            ORDER = [(0, t, mi) for t in range(3) for mi in range(12)]

            def enqueue_chunk(ch_i):
                st = front_alloc(ch_i)
                if st["waff"] is None:
                    st["waff"] = states[ch_i - 1]["waff"]
                states[ch_i] = st
                if ch_i == NCH - 1:
                    # last chunk: v m-tiles are reserved (tag NCH) to feed
                    # its own otherwise-unpumped stage_back
                    queue.extend((ch_i, 0, t, mi)
                                 for t in range(3) for mi in range(8))
                    queue.extend((NCH, 0, t, mi)
                                 for t in range(3) for mi in range(8, 12))
                else:
                    queue.extend((ch_i, kind, a, b) for kind, a, b in ORDER)

            # chunk0 front quarter-major so each wqkv quarter feeds 9
            # consecutive m-tiles (PE outruns the weight DMA otherwise)
            for q4 in range(4):
                for t in range(3):
                    for mi in range(3 * q4, 3 * q4 + 3):
                        qkv_mtile(cur, t, mi)
            if NCH > 1:
                enqueue_chunk(1)

            for ch in range(NCH):
                if ch + 2 < NCH:
                    enqueue_chunk(ch + 2)
                pump_limit[0] = ch + 1 if ch < NCH - 1 else NCH
                stage_back(ch, states[ch], pump)
                # barrier: everything belonging to chunk ch+1 must be issued
                while queue and queue[0][0] == ch + 1:
                    run_item(queue.pop(0))
                del states[ch]

    nc.finalize()
    _NC_CACHE[key] = nc
    return nc


def _prep_weights(inputs):
    in_proj_w = np.asarray(inputs["in_proj_w"], np.float32)
    in_proj_b = np.asarray(inputs["in_proj_b"], np.float32)
    out_proj_w = np.asarray(inputs["out_proj_w"], np.float32)
    out_proj_b = np.asarray(inputs["out_proj_b"], np.float32)
    fus_w1 = np.asarray(inputs["fus_w1"], np.float32)
    fus_b1 = np.asarray(inputs["fus_b1"], np.float32)
    fus_w2 = np.asarray(inputs["fus_w2"], np.float32)
    fus_b2 = np.asarray(inputs["fus_b2"], np.float32)
    ln_g = np.asarray(inputs["ln_g"], np.float32)
    ln_b = np.asarray(inputs["ln_b"], np.float32)
    aff_w = np.asarray(inputs["aff_w"], np.float32)
    aff_b = np.asarray(inputs["aff_b"], np.float32)

    scale = 1.0 / np.sqrt(np.float32(HD))
    W = in_proj_w.copy()
    W[:E] *= scale
    bq = in_proj_b.copy()
    bq[:E] *= scale
    # W.T is [512(k), 1536(m)]; sbuf wants [128, 4(ksub), 1536]
    wqkv_h = np.ascontiguousarray(
        W.T.reshape(4, 128, 1536).transpose(1, 0, 2)).astype(mybir.dt.np(BF16))
    bqkv_h = np.ascontiguousarray(bq.reshape(12, 128).T)

    # fold out_proj into fus_w1; permute (h,d) -> (p, hl, d) to match v layout
    perm = np.empty(E, np.int64)
    for h in range(NH):
        for d in range(HD):
            perm[(h // 2) * 128 + (h % 2) * 64 + d] = h * HD + d
    blocks = []
    for i in range(3):
        blk = fus_w1[:, i * E:(i + 1) * E] @ out_proj_w  # [256, 512]
        blocks.append(blk[:, perm])
    W1o = np.concatenate(blocks, axis=1)  # [256, 1536] cols = (i, p, hl, d)
    w1o_h = np.ascontiguousarray(
        W1o.T.reshape(12, 128, 256).transpose(1, 0, 2)).astype(mybir.dt.np(BF16))
    beff = fus_b1 + fus_w1 @ np.tile(out_proj_b, 3)
    beff_h = np.ascontiguousarray(beff.reshape(2, 128).T)

    w2_h = np.ascontiguousarray(
        fus_w2.T.reshape(2, 128, 512).transpose(1, 0, 2)).astype(mybir.dt.np(BF16))
    b2_h = np.ascontiguousarray(fus_b2.reshape(4, 128).T)

    # ln_g folds into the expert weight columns; ln_b + aff_b fold into the
    # expert bias: aff(LN) = (W_e*diag(g)) @ t + (W_e @ ln_b + b_e)
    A = np.concatenate(
        [aff_w[e].T * ln_g[:, None] for e in range(NE)], axis=1)  # [512, 2048]
    waff_h = np.ascontiguousarray(
        A.reshape(4, 128, 2048).transpose(1, 0, 2)).astype(mybir.dt.np(BF16))
    baff_eff = np.stack([aff_b[e] + aff_w[e] @ ln_b for e in range(NE)])
    baff_h = np.ascontiguousarray(baff_eff.reshape(NE * 4, 128).T)

    # block-diag 64-wide: reduces each head's 64 features and broadcasts the
    # per-head sum back to all 64 slots of that head (per 128-plane)
    sel2_h = np.zeros((128, 128), np.float32)
    for r in range(128):
        for m in range(128):
            if r // 64 == m // 64:
                sel2_h[r, m] = 1.0

    bf = mybir.dt.np(BF16)
    return {
        "wqkv": wqkv_h, "bqkv": bqkv_h, "w1o": w1o_h, "beff": beff_h,
        "w2": w2_h, "b2": b2_h,
        "waff": waff_h, "baff": baff_h,
        "sel2": sel2_h.astype(bf),
        "ones512": np.full((128, 1), 1.0 / E, mybir.dt.np(BF16)),
    }


def kernel(**inputs):
    img = np.asarray(inputs["image_embeddings"], np.float32)
    txt = np.asarray(inputs["text_embeddings"], np.float32)
    kno = np.asarray(inputs["knowledge_embeddings"], np.float32)
    labels = np.asarray(inputs["affective_labels"]).astype(np.int64).ravel()
    assert img.shape == (B, E)

    # ---- host-side expert routing (per-core fixed capacities) ----
    core_idx = np.zeros((NCORES, R), np.int64)
    core_val = np.zeros((NCORES, R), bool)
    for e in range(NE):
        ids = np.nonzero(labels == e)[0]
        assert len(ids) <= NCORES * CAP, f"expert {e} overflow: {len(ids)}"
        parts = np.array_split(ids, NCORES)
        for c in range(NCORES):
            seg = parts[c]
            core_idx[c, e * CAP: e * CAP + len(seg)] = seg
            core_val[c, e * CAP: e * CAP + len(seg)] = True

    wmap = _prep_weights(inputs)

    bf = mybir.dt.np(BF16)
    in_maps = []
    for c in range(NCORES):
        gi = core_idx[c]
        xg = np.stack([img[gi], txt[gi], kno[gi]])        # [3, R, 512]
        xg = xg.transpose(0, 2, 1)                        # [3, 512, R]
        xt_h = np.ascontiguousarray(xg.reshape(3, 4, 128, R)).astype(bf)
        m = dict(wmap)
        m["xt"] = xt_h
        in_maps.append(m)

    nc = _build_program()
    res = run_bass_kernel_spmd(nc, in_maps, core_ids=list(range(NCORES)))
    global LAST_RESULTS, LAST_NC
    LAST_RESULTS = res
    LAST_NC = nc

    out_full = np.zeros((B, E), np.float32)
    for c in range(NCORES):
        oT = res.results[c]["outT"].reshape(E, R).T       # [R, 512]
        v = core_val[c]
        out_full[core_idx[c][v]] = oT[v]
    return out_full


if __name__ == "__main__":
    rng = np.random.default_rng(0)
    fake = {
        "image_embeddings": rng.standard_normal((B, E)).astype(np.float32),
        "text_embeddings": rng.standard_normal((B, E)).astype(np.float32),
        "knowledge_embeddings": rng.standard_normal((B, E)).astype(np.float32),
        "affective_labels": rng.integers(0, NE, B),
        "in_proj_w": (rng.standard_normal((3 * E, E)) * 0.02).astype(np.float32),
        "in_proj_b": np.zeros(3 * E, np.float32),
        "out_proj_w": (rng.standard_normal((E, E)) * 0.02).astype(np.float32),
        "out_proj_b": np.zeros(E, np.float32),
        "fus_w1": (rng.standard_normal((H, 3 * E)) * 0.02).astype(np.float32),
        "fus_b1": np.zeros(H, np.float32),
        "fus_w2": (rng.standard_normal((E, H)) * 0.02).astype(np.float32),
        "fus_b2": np.zeros(E, np.float32),
        "ln_g": np.ones(E, np.float32),
        "ln_b": np.zeros(E, np.float32),
        "aff_w": (rng.standard_normal((NE, E, E)) * 0.02).astype(np.float32),
        "aff_b": np.zeros((NE, E), np.float32),
    }
    out = kernel(**fake)
    print("kernel ran, out:", out.shape, out.dtype, np.abs(out).max())
